# revision 40
# baseline (speedup 1.0000x reference)
"""CrossAttentionBlock Trainium2 kernel (v2).

Math:  q = (Wq xq + bq)/4; k = Wk xkv + bk; v = gamma*(Wv xkv + bv)
       P = softmax_rows(q^T k); out = x_q + v @ P^T   (gamma folded into v)

Strategy (8 cores, sequence-parallel: each core owns NQ=1728 queries vs all
N=13824 keys):
  * Host prep (layouts + the tiny 16xC q/k projections, f32, one fp8
    quantization): q8/k8 in fp8 DoubleRow layout ([8, 2N], virtual row
    r=p+8o, pow2-scaled 2^8/2^6); xkv transposed+tiled to fp8 key-major
    xkvT [128, N] ([key_local, 256s+128o+c]); Wv^T (gamma- and pow2-
    scaled) as a plain fp8 [128,128] stationary; x_q (+gamma*bv) in f32.
  * Device main loop: S^T pair-supertiles (2 key tiles x W query cols in
    one 2-bank PSUM slot, 3-slot ring) via fp8 DoubleRow matmuls; exp with
    deferred normalization, scaled exp(s)/32 so downstream sums stay in
    the e4m3 finite range (max 240).  ACT (table exp, scale/bias APs) and
    DVE (Schraudolph affine-to-uint8 e4m3 bit trick; uint8 saturation maps
    deep-negative scores to +0 instead of fp8 inf/nan) strictly alternate
    pairs - these two engines are the only PSUM readers and bound the
    kernel; regularity beats nominal weighted-RR capacity here.  exp'd
    tiles feed two accumulating DR matmuls: zu = sum_m xkvT_m ex_m (the
    UNPROJECTED attention numerator - no per-tile v evacuation ever) and
    rowsums rs = ones^T ex.  The last chunk (192 cols) runs 4-key-tile
    quad supertiles: one 768-elem exp op per slot amortizes the ~185ns
    fixed PSUM-access cost that dominates small ops.
  * Per-chunk epilogue, pipelined into the next chunk: zu -> fp8 (ACT),
    one plain fp8 [128x128] Wv matmul, evac with 1/sv scale-AP (ACT),
    reciprocal of rs (DVE), broadcast of 1/rs via replicate-DMA (0-stride
    free dim - no engine cost), Pool mul + residual add, store.  The small
    final chunk shortens the serial tail; its broadcast uses PE+DVE
    (lower latency than the replicate-DMA) and one single y DMA.
"""

import contextlib
import math

import numpy as np
import ml_dtypes

import concourse.bass as bass
import concourse.mybir as mybir
from concourse import bacc
from concourse.tile import TileContext
from concourse.bass_utils import run_bass_kernel_spmd

# The act-table placement pass resolves each activation to the first set
# containing its function, which splits {Exp, Identity/Copy} (set 0) from Ln
# (set 6) and thrashes ~1.3us LoadActFuncSet swaps mid-kernel.  This kernel
# only uses funcs that all live in natural_log_exp_and_others, so replace
# the pass with one pre-placed load of that set (walrus adopts pre-placed
# loads); set id keeps its original index so the walrus mapping is intact.
_orig_gat = bacc.get_activation_tables
_orig_iatl = getattr(bacc.Bacc.insert_act_table_loads, "_nl_orig",
                     bacc.Bacc.insert_act_table_loads)


def _single_act_load(self):
    used = {i.func for b in self.main_func.blocks
            for i in b.instructions if isinstance(i, mybir.InstActivation)}
    if not used:
        return
    tabs = _orig_gat(self.m.arch)
    names = list(tabs.keys())
    keep = "natural_log_exp_and_others"
    if keep not in tabs or not used <= tabs[keep]:
        return _orig_iatl(self)       # fall back to the stock pass
    ld = mybir.InstLoadActFuncSet(
        name=self.get_next_instruction_name(), ins=[], outs=[],
        act_func_set_id=names.index(keep))
    ld.engine = mybir.EngineType.Activation
    self.register_instruction(ld)
    # place directly before the first activation (same block) so it rides
    # the ACT queue during the input-DMA window instead of delaying any
    # block-entry barrier
    for blk in self.main_func.blocks:
        for idx, inst in enumerate(blk.instructions):
            if isinstance(inst, mybir.InstActivation):
                blk.instructions.insert(idx, ld)
                return
    self.main_func.blocks[0].instructions.insert(0, ld)


_single_act_load._nl_orig = _orig_iatl
bacc.Bacc.insert_act_table_loads = _single_act_load

F32 = mybir.dt.float32
BF16 = mybir.dt.bfloat16
FP8 = mybir.dt.float8e4
U8 = mybir.dt.uint8
AF = mybir.ActivationFunctionType
DR = mybir.MatmulPerfMode.DoubleRow

C = 128
RC = 16
D = H = W = 24
N = D * H * W            # 13824 keys
NCORES = 8
NQ = N // NCORES         # 1728 queries per core
MT = N // 128            # 108 key tiles
PAIRS = MT // 2          # 54 key-tile pairs
LAGP = 7                 # zu matmuls trail exp by this many pairs
RLAG = 13                # rs matmuls trail further: frees the rs bank later so
                         # the ACT-side rs evacuation can be spaced away from
                         # the other epilogue injections
CHW = [512, 512, 512, 192]
COFF = [0, 512, 1024, 1536]

SQ = 256.0               # q fp8 pow2 scale
SK = 64.0                # k fp8 pow2 scale
ES = 1.0 / (SQ * SK)     # exp input scale
LNDIV = math.log(32.0)   # ex = exp(s)/32 keeps zu inside fp8 range (max 240)
LOG2E = 1.4426950408889634
EXP8_SCALE = 8.0 * LOG2E
DVE_SCALE = EXP8_SCALE * ES
DVE_BIAS = 56.0 - 0.3 - 40.0   # e4m3 Schraudolph bias, -40 = the /32
# exp pair -> engine split.  Strict A,D,A,D alternation is the unique
# stall-free pattern on the 3-slot PSUM ring: same-engine ops land at slot
# distance 2 (never 3), so the exp(s) -> PE S^T(s+3) -> exp(s+3) rewrite
# chain (~350 ns) stays off every engine's back-to-back path.  The cycle is
# then 2 slots per max(ACT 1038, DVE 1192) = DVE-bound; ACT's 154 ns/period
# slack absorbs all epilogue PSUM reads (spaced injections below).
ACT_FRAC = [0.50, 0.50, 0.50, 0.50]

_BUILD_CACHE: dict = {}


def _bcast_ap(src):
    """[1, w] AP -> [1, 128, w] with a 0-stride repeat dim (DMA replicate)."""
    ap = list(src.ap)
    return bass.AP(src.tensor, src.offset, [ap[0]] + [[0, 128]] + ap[1:])


def build_nc(repeats: int = 1):
    key = repeats
    if key in _BUILD_CACHE:
        return _BUILD_CACHE[key]

    nc = bacc.Bacc("TRN2", target_bir_lowering=False, debug=False,
                   num_devices=NCORES)
    # qk: [q_db (2*NQ) | k slice0 (2*2048) | k slice1 (2*(N-2048))], all in
    # DoubleRow o-major halves; one DMA covers q + the first 16 key tiles
    qk_dr = nc.dram_tensor("qk", [8, 2 * NQ + 2 * N], FP8,
                           kind="ExternalInput").ap()
    # v8T: host-projected values gamma*(Wv@xkv), pow2-scaled to fp8,
    # key-major tiled like the old xkvT.  Folding Wv on the host removes the
    # per-chunk wv matmul + zs/po evacuations AND one fp8 quantization.
    v8T_dr = nc.dram_tensor("v8T", [C, N], FP8, kind="ExternalInput").ap()
    lnisv_dr = nc.dram_tensor("lnisv", [1, 1], F32, kind="ExternalInput").ap()
    isv1_dr = nc.dram_tensor("isv1", [1, 1], F32, kind="ExternalInput").ap()
    xq_dr = nc.dram_tensor("xq32", [C, NQ], F32, kind="ExternalInput").ap()
    y = nc.dram_tensor("y", [C, NQ], F32, kind="ExternalOutput").ap()

    with TileContext(nc) as tc, contextlib.ExitStack() as ctx:
        cpool = ctx.enter_context(tc.tile_pool(name="consts", bufs=1))
        ppool = ctx.enter_context(tc.tile_pool(name="psum", bufs=1, space="PSUM"))
        spool = ctx.enter_context(tc.tile_pool(name="work", bufs=1))

        # ---- input DMAs, critical-path first ---------------------------------
        qk_sb = cpool.tile([8, 2 * NQ + 2 * N], FP8)
        Q0 = 2 * NQ          # 3456
        K0 = Q0 + 2 * 2048   # end of k slice0
        nc.sync.dma_start(qk_sb[:, 0:K0], qk_dr[:, 0:K0])
        nc.sync.dma_start(qk_sb[:, K0:], qk_dr[:, K0:])
        v8T = cpool.tile([C, N], FP8)
        for qq in range(4):
            sl = bass.ts(qq, N // 4)
            nc.sync.dma_start(v8T[:, sl], v8T_dr[:, sl])
        lnisv = cpool.tile([1, 1], F32)
        nc.sync.dma_start(lnisv[:], lnisv_dr[:])
        isv1 = cpool.tile([1, 1], F32)
        nc.sync.dma_start(isv1[:], isv1_dr[:])
        xq_sb = cpool.tile([C, NQ], F32)
        nc.sync.dma_start(xq_sb[:], xq_dr[:])

        ones_db = cpool.tile([C, 32], FP8)
        nc.gpsimd.memset(ones_db[:], 1.0)
        ones_row = cpool.tile([1, C], BF16)
        nc.gpsimd.memset(ones_row[:], 1.0)
        exp_bias = cpool.tile([C, 1], F32)
        nc.gpsimd.memset(exp_bias[:], -LNDIV)
        exp_scale = cpool.tile([C, 1], F32)
        nc.gpsimd.memset(exp_scale[:], ES)
        # dummy exp: hoists the ~1.3us activation-table load into the input
        # DMA window instead of delaying the first real ACT exp
        warm = cpool.tile([C, 1], F32)
        nc.scalar.activation(warm[:], exp_scale[:], AF.Exp)

        q3 = qk_sb[:, 0:Q0].rearrange("p (o x) -> p o x", o=2)
        k3a = qk_sb[:, Q0:K0].rearrange("p (o x) -> p o x", o=2)
        k3b = qk_sb[:, K0:].rearrange("p (o x) -> p o x", o=2)

        def k3t(t):
            if t < 16:
                return k3a[:, :, bass.ts(t, 128)]
            return k3b[:, :, bass.ts(t - 16, 128)]
        ones3 = ones_db.rearrange("p (b c) -> p b c", b=2)[:, :, 0:1]

        # ---- pipelined epilogue steps (run inside the NEXT chunk) ------------
        # All PSUM-reading epilogue ops go to ACT (DVE stays pure-exp): the
        # zu evac (now the final projected numerator, f32 straight to SBUF)
        # and the rowsum reciprocal via ln -> exp(-ln - ln sv), whose ln
        # doubles as the rs evacuation and whose bias folds in the host-side
        # fp8 scale.  Injections are spaced ~6 ups apart so ACT's per-period
        # slack covers them; the latency-critical tail keeps the DVE recip +
        # PE broadcast instead.
        pend = {}

        def epi_zu():
            zu, ch = pend.pop("zu_p")
            w = CHW[ch]
            ou = spool.tile([C, 512], F32, tag="outus", bufs=2)
            nc.scalar.copy(ou[:, 0:w], zu[:, 0:w])
            pend["outu_s"] = (ou, ch)

        def epi_rsln():
            rs, ch = pend.pop("rs_p")
            w = CHW[ch]
            rsl = spool.tile([1, 512], F32, tag="rsl", bufs=2)
            nc.scalar.activation(rsl[:, 0:w], rs[:, 0:w], AF.Ln)
            pend["rs_l"] = (rsl, ch)

        def epi_rsexp():
            rsl, ch = pend.pop("rs_l")
            w = CHW[ch]
            rsb = spool.tile([1, 512], F32, tag="rsb", bufs=2)
            nc.scalar.activation(rsb[:, 0:w], rsl[:, 0:w], AF.Exp,
                                 scale=-1.0, bias=lnisv[0:1, :])
            bc = spool.tile([C, 512], F32, tag="bc", bufs=2)
            nc.sync.dma_start(bc[:, 0:w], _bcast_ap(rsb[0:1, 0:w]))
            pend["bc"] = (bc, ch)

        def epi_recip_tail():
            rs, ch = pend.pop("rs_p")
            w = CHW[ch]
            recip = spool.tile([1, 512], F32, tag="recip", bufs=1)
            nc.vector.reciprocal_approx_fast(out=recip[:, 0:w], in_=rs[:, 0:w])
            # latency-critical tail: PE broadcast instead of replicate-DMA;
            # the rb copy folds in the 1/sv fp8 descale.  The broadcast PSUM
            # tile is consumed directly by the DVE fin mult - no bc copy.
            rb = spool.tile([1, 512], BF16, tag="rb", bufs=1)
            nc.gpsimd.tensor_scalar(out=rb[:, 0:w], in0=recip[:, 0:w],
                                    scalar1=isv1[0:1, :], scalar2=None,
                                    op0=mybir.AluOpType.mult)
            bp = ppool.tile([C, 1024], F32, tag="st", bufs=3)
            nc.tensor.matmul(bp[:, 0:w], ones_row[:], rb[:, 0:w],
                             start=True, stop=True)
            pend["bc_p"] = (bp, ch)

        def epi_fin(halves=1):
            ou, ch = pend.pop("outu_s")
            bc, _ = pend.pop("bc")
            w, off = CHW[ch], COFF[ch]
            hw = w // halves
            for h in range(halves):
                hsl = slice(h * hw, (h + 1) * hw)
                t1 = spool.tile([C, 512], F32, tag="t1", bufs=2)
                nc.gpsimd.tensor_mul(t1[:, 0:hw], ou[:, hsl], bc[:, hsl])
                res = spool.tile([C, 512], F32, tag="res", bufs=2)
                nc.gpsimd.tensor_add(res[:, 0:hw], t1[:, 0:hw],
                                     xq_sb[:, off + h * hw:off + (h + 1) * hw])
                nc.sync.dma_start(y[:, off + h * hw:off + (h + 1) * hw],
                                  res[:, 0:hw])

        def epi_fin_tail(halves=2):
            ou, ch = pend.pop("outu_s")
            bp, _ = pend.pop("bc_p")
            w, off = CHW[ch], COFF[ch]
            hw = w // halves
            for h in range(halves):
                hsl = slice(h * hw, (h + 1) * hw)
                t1 = spool.tile([C, 512], F32, tag="t1", bufs=2)
                nc.vector.tensor_mul(t1[:, 0:hw], ou[:, hsl], bp[:, hsl])
                res = spool.tile([C, 512], F32, tag="res", bufs=2)
                nc.gpsimd.tensor_add(res[:, 0:hw], t1[:, 0:hw],
                                     xq_sb[:, off + h * hw:off + (h + 1) * hw])
                nc.sync.dma_start(y[:, off + h * hw:off + (h + 1) * hw],
                                  res[:, 0:hw])

        # ---- main loop -------------------------------------------------------
        def do_exp(ua, ex_v, st_v):
            if ua:
                nc.scalar.activation(ex_v, st_v, AF.Exp,
                                     bias=exp_bias[:], scale=exp_scale[:])
            else:
                # uint8 out: conversion saturates at 0, so deep negative
                # scores clamp to fp8 +0 instead of the e4m3 inf/nan
                # patterns (bytes 0xF8..0xFF)
                nc.vector.tensor_scalar(
                    out=ex_v.bitcast(U8), in0=st_v,
                    scalar1=DVE_SCALE, scalar2=DVE_BIAS,
                    op0=mybir.AluOpType.mult, op1=mybir.AluOpType.add)

        # One flat loop over the global pair index: PE interleaves chunk c's
        # trailing zu/rs with chunk c+1's leading S^T so the exp ring never
        # drains at chunk boundaries.  zu/rs emission ups are capped near the
        # chunk end so each accumulator bank's last matmul lands just before
        # its ACT evacuation (up 3 / 11 of the next chunk) and the bank is
        # back in service for the next chunk's first accumulation (up 7 / 13).
        # zu/rs emission schedule.  Mid-chunks spread the 54 accumulation
        # matmuls linearly (~1.1 per up, inside PE's per-period slack) so
        # there is never a PE burst that stalls S^T production, while each
        # chunk's stream ends early enough in the next chunk (up 6 / 12) for
        # the ACT evacuations (zu copy up 8, rs ln up 14) to turn the bank
        # around before the next stream starts (up 11 / 17).  The final
        # chunk compresses instead: the exp engines are draining, so PE
        # bursts are free and the tail shortens.
        EXBUFS = 20              # ex ring depth >= max rs lag (17) + 3
        import collections as _cl
        sched_zu = _cl.defaultdict(list)
        sched_rs = _cl.defaultdict(list)
        for c in range(4):
            for s in range(PAIRS):
                if c < 3:
                    uz = 11 + (s * 49) // 53
                    ur = 17 + (s * 49) // 53
                else:
                    # steeper spread: still floored past the bank handoff,
                    # but ending by up 55/56 to shorten the drain tail
                    uz = 11 + (s * 43) // 53
                    ur = 17 + (s * 38) // 53
                sched_zu[PAIRS * c + uz].append((c, s))
                sched_rs[PAIRS * c + ur].append((c, s))

        zu_tiles, rs_tiles, ex_tiles, accs = {}, {}, {}, [0.0] * 4
        TOT = 4 * PAIRS
        for g in range(TOT + RLAG):
            c1, up = divmod(g, PAIRS)
            if c1 < 4:
                if up == 8 and "zu_p" in pend:
                    epi_zu()
                if up == 14 and "rs_p" in pend:
                    epi_rsln()
                if up == 20 and "rs_l" in pend:
                    epi_rsexp()
                if up == 26 and "outu_s" in pend and "bc" in pend:
                    epi_fin()
            if g < TOT:
                ch, s = c1, up
                w, off = CHW[ch], COFF[ch]
                quad = w <= 256
                if not quad or s % 2 == 0:
                    accs[ch] += ACT_FRAC[ch]
                    ua = accs[ch] >= 1.0
                    if ua:
                        accs[ch] -= 1.0
                if not quad:
                    stp = ppool.tile([C, 1024], F32, tag="st", bufs=3)
                    for j in range(2):
                        nc.tensor.matmul(stp[:, w * j:w * j + w],
                                         k3t(2 * s + j),
                                         q3[:, :, bass.ds(off, w)],
                                         start=True, stop=True,
                                         perf_mode=DR)
                    ex = spool.tile([C, 1024], FP8, tag="ex",
                                    bufs=EXBUFS)
                    do_exp(ua, ex[:, 0:2 * w], stp[:, 0:2 * w])
                    ex_tiles[(ch, s)] = (ex, 0)
                elif s % 2 == 0:
                    # quad: key tiles 2s..2s+3 in one slot, banks at
                    # [0:2w] and [512:512+2w]; single exp op of 4w elems
                    stp = ppool.tile([C, 1024], F32, tag="st", bufs=3)
                    for j in range(4):
                        base = w * j if j < 2 else 512 + w * (j - 2)
                        nc.tensor.matmul(stp[:, base:base + w],
                                         k3t(2 * s + j),
                                         q3[:, :, bass.ds(off, w)],
                                         start=True, stop=True,
                                         perf_mode=DR)
                    ex = spool.tile([C, 1024], FP8, tag="ex",
                                    bufs=EXBUFS)
                    st_v = stp.rearrange(
                        "p (b x) -> p b x", b=2)[:, :, 0:2 * w]
                    ex_v = ex[:, 0:4 * w].rearrange(
                        "p (b x) -> p b x", b=2)
                    do_exp(ua, ex_v, st_v)
                    ex_tiles[(ch, s)] = (ex, 0)
                    ex_tiles[(ch, s + 1)] = (ex, 2 * w)
            for (c, s) in sched_zu.get(g, []):
                w = CHW[c]
                if s == 0:
                    zu_tiles[c] = ppool.tile([C, 512], F32, tag="zu", name="zu")
                ex, xoff = ex_tiles[(c, s)]
                ex3 = ex[:, xoff:xoff + 2 * w].rearrange(
                    "p (b x) -> p b x", b=2)
                xk3 = v8T[:, bass.ds(256 * s, 256)].rearrange(
                    "p (b c) -> p b c", b=2)
                nc.tensor.matmul(zu_tiles[c][:, 0:w], xk3, ex3, perf_mode=DR,
                                 start=(s == 0), stop=(s == PAIRS - 1))
                if s == PAIRS - 1:
                    pend["zu_p"] = (zu_tiles[c], c)
            for (c, s) in sched_rs.get(g, []):
                w = CHW[c]
                if s == 0:
                    rs_tiles[c] = ppool.tile([1, 512], F32, tag="rs", name="rs")
                ex, xoff = ex_tiles.pop((c, s))
                ex3 = ex[:, xoff:xoff + 2 * w].rearrange(
                    "p (b x) -> p b x", b=2)
                nc.tensor.matmul(rs_tiles[c][:, 0:w], ones3, ex3,
                                 perf_mode=DR,
                                 start=(s == 0), stop=(s == PAIRS - 1))
                if s == PAIRS - 1:
                    pend["rs_p"] = (rs_tiles[c], c)

        # tail: final chunk's epilogue.  Per-engine issue order is what
        # matters: ACT zs -> ou, PE wv -> broadcast, DVE recip -> bc copy,
        # Pool rb -> fin; one y DMA (fixed HWDGE+latency cost dominates the
        # 192-col transfer).
        epi_zu()
        epi_recip_tail()
        epi_fin_tail(halves=2)

    nc.compile()
    _BUILD_CACHE[key] = nc
    return nc


def _prep_in_maps(x_q, x_kv, Wq, bq, Wk, bk, Wv, bv, gamma):
    f8 = ml_dtypes.float8_e4m3
    f32 = np.float32
    xq = np.asarray(x_q, f32).reshape(C, N)
    xkv = np.asarray(x_kv, f32).reshape(C, N)
    Wq = np.asarray(Wq, f32)
    bq = np.asarray(bq, f32)
    Wk = np.asarray(Wk, f32)
    bk = np.asarray(bk, f32)
    Wv = np.asarray(Wv, f32)
    bv = np.asarray(bv, f32)
    gamma = float(np.asarray(gamma, f32).reshape(()))

    # q/k projections (16xC) in f32 on host, straight into fp8 DR layout
    q = (Wq @ xq + bq[:, None]) * (0.25 * SQ)
    k = (Wk @ xkv + bk[:, None]) * SK
    q8 = np.clip(q, -224, 224).astype(f8)
    k8 = np.clip(k, -224, 224).astype(f8)
    k_s0 = np.concatenate([k8[0:8, 0:2048], k8[8:16, 0:2048]], axis=1)
    k_s1 = np.concatenate([k8[0:8, 2048:], k8[8:16, 2048:]], axis=1)

    # host-projected values: one fp8 quantization of gamma*(Wv@xkv) instead
    # of quantizing xkv AND Wv separately
    v_host = (Wv @ xkv) * gamma              # [C, N] f32
    am = float(np.abs(v_host).max())
    sv = float(2.0 ** np.floor(np.log2(224.0 / am))) if am > 0 else 1.0
    sv = min(max(sv, 2.0 ** -20), 2.0 ** 20)
    v8 = np.clip(v_host * sv, -224, 224).astype(f8)
    v8T = np.ascontiguousarray(
        v8.reshape(C, MT, 128).transpose(2, 1, 0).reshape(128, N))
    lnisv = np.full((1, 1), -math.log(sv), f32)
    isv1 = np.full((1, 1), 1.0 / sv, f32)
    resid = gamma * bv  # softmax rows sum to 1

    in_maps = []
    for c in range(NCORES):
        sl = slice(c * NQ, (c + 1) * NQ)
        q8c = q8[:, sl]
        qk = np.ascontiguousarray(np.concatenate(
            [q8c[0:8], q8c[8:16], k_s0, k_s1], axis=1))
        in_maps.append({
            "qk": qk, "v8T": v8T, "lnisv": lnisv, "isv1": isv1,
            "xq32": np.ascontiguousarray(xq[:, sl] + resid[:, None]),
        })
    return in_maps


def kernel(x_q, x_kv, Wq, bq, Wk, bk, Wv, bv, gamma):
    nc = build_nc(repeats=1)
    in_maps = _prep_in_maps(x_q, x_kv, Wq, bq, Wk, bk, Wv, bv, gamma)
    res = run_bass_kernel_spmd(nc, in_maps, list(range(NCORES)))
    out = np.concatenate([res.results[c]["y"] for c in range(NCORES)], axis=1)
    return out.reshape(1, C, D, H, W).astype(np.float32)



# revision 41
# speedup vs baseline: 1.0010x; 1.0010x over previous
"""CrossAttentionBlock Trainium2 kernel (v2).

Math:  q = (Wq xq + bq)/4; k = Wk xkv + bk; v = gamma*(Wv xkv + bv)
       P = softmax_rows(q^T k); out = x_q + v @ P^T   (gamma folded into v)

Strategy (8 cores, sequence-parallel: each core owns NQ=1728 queries vs all
N=13824 keys):
  * Host prep (layouts + the tiny 16xC q/k projections, f32, one fp8
    quantization): q8/k8 in fp8 DoubleRow layout ([8, 2N], virtual row
    r=p+8o, pow2-scaled 2^8/2^6); xkv transposed+tiled to fp8 key-major
    xkvT [128, N] ([key_local, 256s+128o+c]); Wv^T (gamma- and pow2-
    scaled) as a plain fp8 [128,128] stationary; x_q (+gamma*bv) in f32.
  * Device main loop: S^T pair-supertiles (2 key tiles x W query cols in
    one 2-bank PSUM slot, 3-slot ring) via fp8 DoubleRow matmuls; exp with
    deferred normalization, scaled exp(s)/32 so downstream sums stay in
    the e4m3 finite range (max 240).  ACT (table exp, scale/bias APs) and
    DVE (Schraudolph affine-to-uint8 e4m3 bit trick; uint8 saturation maps
    deep-negative scores to +0 instead of fp8 inf/nan) strictly alternate
    pairs - these two engines are the only PSUM readers and bound the
    kernel; regularity beats nominal weighted-RR capacity here.  exp'd
    tiles feed two accumulating DR matmuls: zu = sum_m xkvT_m ex_m (the
    UNPROJECTED attention numerator - no per-tile v evacuation ever) and
    rowsums rs = ones^T ex.  The last chunk (192 cols) runs 4-key-tile
    quad supertiles: one 768-elem exp op per slot amortizes the ~185ns
    fixed PSUM-access cost that dominates small ops.
  * Per-chunk epilogue, pipelined into the next chunk: zu -> fp8 (ACT),
    one plain fp8 [128x128] Wv matmul, evac with 1/sv scale-AP (ACT),
    reciprocal of rs (DVE), broadcast of 1/rs via replicate-DMA (0-stride
    free dim - no engine cost), Pool mul + residual add, store.  The small
    final chunk shortens the serial tail; its broadcast uses PE+DVE
    (lower latency than the replicate-DMA) and one single y DMA.
"""

import contextlib
import math

import numpy as np
import ml_dtypes

import concourse.bass as bass
import concourse.mybir as mybir
from concourse import bacc
from concourse.tile import TileContext
from concourse.bass_utils import run_bass_kernel_spmd

# The act-table placement pass resolves each activation to the first set
# containing its function, which splits {Exp, Identity/Copy} (set 0) from Ln
# (set 6) and thrashes ~1.3us LoadActFuncSet swaps mid-kernel.  This kernel
# only uses funcs that all live in natural_log_exp_and_others, so replace
# the pass with one pre-placed load of that set (walrus adopts pre-placed
# loads); set id keeps its original index so the walrus mapping is intact.
_orig_gat = bacc.get_activation_tables
_orig_iatl = getattr(bacc.Bacc.insert_act_table_loads, "_nl_orig",
                     bacc.Bacc.insert_act_table_loads)


def _single_act_load(self):
    used = {i.func for b in self.main_func.blocks
            for i in b.instructions if isinstance(i, mybir.InstActivation)}
    if not used:
        return
    tabs = _orig_gat(self.m.arch)
    names = list(tabs.keys())
    keep = "natural_log_exp_and_others"
    if keep not in tabs or not used <= tabs[keep]:
        return _orig_iatl(self)       # fall back to the stock pass
    ld = mybir.InstLoadActFuncSet(
        name=self.get_next_instruction_name(), ins=[], outs=[],
        act_func_set_id=names.index(keep))
    ld.engine = mybir.EngineType.Activation
    self.register_instruction(ld)
    # place directly before the first activation (same block) so it rides
    # the ACT queue during the input-DMA window instead of delaying any
    # block-entry barrier
    for blk in self.main_func.blocks:
        for idx, inst in enumerate(blk.instructions):
            if isinstance(inst, mybir.InstActivation):
                blk.instructions.insert(idx, ld)
                return
    self.main_func.blocks[0].instructions.insert(0, ld)


_single_act_load._nl_orig = _orig_iatl
bacc.Bacc.insert_act_table_loads = _single_act_load

F32 = mybir.dt.float32
BF16 = mybir.dt.bfloat16
FP8 = mybir.dt.float8e4
U8 = mybir.dt.uint8
AF = mybir.ActivationFunctionType
DR = mybir.MatmulPerfMode.DoubleRow

C = 128
RC = 16
D = H = W = 24
N = D * H * W            # 13824 keys
NCORES = 8
NQ = N // NCORES         # 1728 queries per core
MT = N // 128            # 108 key tiles
PAIRS = MT // 2          # 54 key-tile pairs
LAGP = 7                 # zu matmuls trail exp by this many pairs
RLAG = 13                # rs matmuls trail further: frees the rs bank later so
                         # the ACT-side rs evacuation can be spaced away from
                         # the other epilogue injections
CHW = [512, 512, 512, 192]
COFF = [0, 512, 1024, 1536]

SQ = 256.0               # q fp8 pow2 scale
SK = 64.0                # k fp8 pow2 scale
ES = 1.0 / (SQ * SK)     # exp input scale
LNDIV = math.log(32.0)   # ex = exp(s)/32 keeps zu inside fp8 range (max 240)
LOG2E = 1.4426950408889634
EXP8_SCALE = 8.0 * LOG2E
DVE_SCALE = EXP8_SCALE * ES
DVE_BIAS = 56.0 - 0.3 - 40.0   # e4m3 Schraudolph bias, -40 = the /32
# exp pair -> engine split.  Strict A,D,A,D alternation is the unique
# stall-free pattern on the 3-slot PSUM ring: same-engine ops land at slot
# distance 2 (never 3), so the exp(s) -> PE S^T(s+3) -> exp(s+3) rewrite
# chain (~350 ns) stays off every engine's back-to-back path.  The cycle is
# then 2 slots per max(ACT 1038, DVE 1192) = DVE-bound; ACT's 154 ns/period
# slack absorbs all epilogue PSUM reads (spaced injections below).
ACT_FRAC = [0.50, 0.50, 0.50, 0.50]

_BUILD_CACHE: dict = {}


def _bcast_ap(src):
    """[1, w] AP -> [1, 128, w] with a 0-stride repeat dim (DMA replicate)."""
    ap = list(src.ap)
    return bass.AP(src.tensor, src.offset, [ap[0]] + [[0, 128]] + ap[1:])


def build_nc(repeats: int = 1):
    key = repeats
    if key in _BUILD_CACHE:
        return _BUILD_CACHE[key]

    nc = bacc.Bacc("TRN2", target_bir_lowering=False, debug=False,
                   num_devices=NCORES)
    # qk: [q_db (2*NQ) | k slice0 (2*2048) | k slice1 (2*(N-2048))], all in
    # DoubleRow o-major halves; one DMA covers q + the first 16 key tiles
    qk_dr = nc.dram_tensor("qk", [8, 2 * NQ + 2 * N], FP8,
                           kind="ExternalInput").ap()
    # v8T: host-projected values gamma*(Wv@xkv), pow2-scaled to fp8,
    # key-major tiled like the old xkvT.  Folding Wv on the host removes the
    # per-chunk wv matmul + zs/po evacuations AND one fp8 quantization.
    v8T_dr = nc.dram_tensor("v8T", [C, N], FP8, kind="ExternalInput").ap()
    lnisv_dr = nc.dram_tensor("lnisv", [1, 1], F32, kind="ExternalInput").ap()
    isv1_dr = nc.dram_tensor("isv1", [1, 1], F32, kind="ExternalInput").ap()
    xq_dr = nc.dram_tensor("xq32", [C, NQ], F32, kind="ExternalInput").ap()
    y = nc.dram_tensor("y", [C, NQ], F32, kind="ExternalOutput").ap()

    with TileContext(nc) as tc, contextlib.ExitStack() as ctx:
        cpool = ctx.enter_context(tc.tile_pool(name="consts", bufs=1))
        ppool = ctx.enter_context(tc.tile_pool(name="psum", bufs=1, space="PSUM"))
        spool = ctx.enter_context(tc.tile_pool(name="work", bufs=1))

        # ---- input DMAs, critical-path first ---------------------------------
        qk_sb = cpool.tile([8, 2 * NQ + 2 * N], FP8)
        Q0 = 2 * NQ          # 3456
        K0 = Q0 + 2 * 2048   # end of k slice0
        nc.sync.dma_start(qk_sb[:, 0:K0], qk_dr[:, 0:K0])
        nc.sync.dma_start(qk_sb[:, K0:], qk_dr[:, K0:])
        v8T = cpool.tile([C, N], FP8)
        for qq in range(4):
            sl = bass.ts(qq, N // 4)
            nc.sync.dma_start(v8T[:, sl], v8T_dr[:, sl])
        lnisv = cpool.tile([1, 1], F32)
        nc.sync.dma_start(lnisv[:], lnisv_dr[:])
        isv1 = cpool.tile([1, 1], F32)
        nc.sync.dma_start(isv1[:], isv1_dr[:])
        xq_sb = cpool.tile([C, NQ], F32)
        nc.sync.dma_start(xq_sb[:], xq_dr[:])

        ones_db = cpool.tile([C, 32], FP8)
        nc.gpsimd.memset(ones_db[:], 1.0)
        ones_row = cpool.tile([1, C], BF16)
        nc.gpsimd.memset(ones_row[:], 1.0)
        exp_bias = cpool.tile([C, 1], F32)
        nc.gpsimd.memset(exp_bias[:], -LNDIV)
        exp_scale = cpool.tile([C, 1], F32)
        nc.gpsimd.memset(exp_scale[:], ES)
        # dummy exp: hoists the ~1.3us activation-table load into the input
        # DMA window instead of delaying the first real ACT exp
        warm = cpool.tile([C, 1], F32)
        nc.scalar.activation(warm[:], exp_scale[:], AF.Exp)

        q3 = qk_sb[:, 0:Q0].rearrange("p (o x) -> p o x", o=2)
        k3a = qk_sb[:, Q0:K0].rearrange("p (o x) -> p o x", o=2)
        k3b = qk_sb[:, K0:].rearrange("p (o x) -> p o x", o=2)

        def k3t(t):
            if t < 16:
                return k3a[:, :, bass.ts(t, 128)]
            return k3b[:, :, bass.ts(t - 16, 128)]
        ones3 = ones_db.rearrange("p (b c) -> p b c", b=2)[:, :, 0:1]

        # ---- pipelined epilogue steps (run inside the NEXT chunk) ------------
        # All PSUM-reading epilogue ops go to ACT (DVE stays pure-exp): the
        # zu evac (now the final projected numerator, f32 straight to SBUF)
        # and the rowsum reciprocal via ln -> exp(-ln - ln sv), whose ln
        # doubles as the rs evacuation and whose bias folds in the host-side
        # fp8 scale.  Injections are spaced ~6 ups apart so ACT's per-period
        # slack covers them; the latency-critical tail keeps the DVE recip +
        # PE broadcast instead.
        pend = {}

        def epi_zu():
            zu, ch = pend.pop("zu_p")
            w = CHW[ch]
            ou = spool.tile([C, 512], F32, tag="outus", bufs=2)
            nc.scalar.copy(ou[:, 0:w], zu[:, 0:w])
            pend["outu_s"] = (ou, ch)

        def epi_rsln():
            rs, ch = pend.pop("rs_p")
            w = CHW[ch]
            rsl = spool.tile([1, 512], F32, tag="rsl", bufs=2)
            nc.scalar.activation(rsl[:, 0:w], rs[:, 0:w], AF.Ln)
            pend["rs_l"] = (rsl, ch)

        def epi_rsexp():
            rsl, ch = pend.pop("rs_l")
            w = CHW[ch]
            rsb = spool.tile([1, 512], F32, tag="rsb", bufs=2)
            nc.scalar.activation(rsb[:, 0:w], rsl[:, 0:w], AF.Exp,
                                 scale=-1.0, bias=lnisv[0:1, :])
            bc = spool.tile([C, 512], F32, tag="bc", bufs=2)
            nc.sync.dma_start(bc[:, 0:w], _bcast_ap(rsb[0:1, 0:w]))
            pend["bc"] = (bc, ch)

        def epi_recip_tail():
            rs, ch = pend.pop("rs_p")
            w = CHW[ch]
            recip = spool.tile([1, 512], F32, tag="recip", bufs=1)
            nc.vector.reciprocal_approx_fast(out=recip[:, 0:w], in_=rs[:, 0:w])
            # latency-critical tail: PE broadcast instead of replicate-DMA;
            # the rb copy folds in the 1/sv fp8 descale.  The broadcast PSUM
            # tile is consumed directly by the DVE fin mult - no bc copy.
            rb = spool.tile([1, 512], BF16, tag="rb", bufs=1)
            nc.gpsimd.tensor_scalar(out=rb[:, 0:w], in0=recip[:, 0:w],
                                    scalar1=isv1[0:1, :], scalar2=None,
                                    op0=mybir.AluOpType.mult)
            bp = ppool.tile([C, 1024], F32, tag="st", bufs=3)
            nc.tensor.matmul(bp[:, 0:w], ones_row[:], rb[:, 0:w],
                             start=True, stop=True)
            pend["bc_p"] = (bp, ch)

        def epi_fin(halves=1):
            ou, ch = pend.pop("outu_s")
            bc, _ = pend.pop("bc")
            w, off = CHW[ch], COFF[ch]
            hw = w // halves
            for h in range(halves):
                hsl = slice(h * hw, (h + 1) * hw)
                t1 = spool.tile([C, 512], F32, tag="t1", bufs=2)
                nc.gpsimd.tensor_mul(t1[:, 0:hw], ou[:, hsl], bc[:, hsl])
                res = spool.tile([C, 512], F32, tag="res", bufs=2)
                nc.gpsimd.tensor_add(res[:, 0:hw], t1[:, 0:hw],
                                     xq_sb[:, off + h * hw:off + (h + 1) * hw])
                nc.sync.dma_start(y[:, off + h * hw:off + (h + 1) * hw],
                                  res[:, 0:hw])

        def epi_fin_tail(halves=2):
            ou, ch = pend.pop("outu_s")
            bp, _ = pend.pop("bc_p")
            w, off = CHW[ch], COFF[ch]
            hw = w // halves
            for h in range(halves):
                hsl = slice(h * hw, (h + 1) * hw)
                t1 = spool.tile([C, 512], F32, tag="t1", bufs=2)
                nc.vector.tensor_mul(t1[:, 0:hw], ou[:, hsl], bp[:, hsl])
                res = spool.tile([C, 512], F32, tag="res", bufs=2)
                nc.gpsimd.tensor_add(res[:, 0:hw], t1[:, 0:hw],
                                     xq_sb[:, off + h * hw:off + (h + 1) * hw])
                nc.sync.dma_start(y[:, off + h * hw:off + (h + 1) * hw],
                                  res[:, 0:hw])

        # ---- main loop -------------------------------------------------------
        def do_exp(ua, ex_v, st_v):
            if ua:
                nc.scalar.activation(ex_v, st_v, AF.Exp,
                                     bias=exp_bias[:], scale=exp_scale[:])
            else:
                # uint8 out: conversion saturates at 0, so deep negative
                # scores clamp to fp8 +0 instead of the e4m3 inf/nan
                # patterns (bytes 0xF8..0xFF)
                nc.vector.tensor_scalar(
                    out=ex_v.bitcast(U8), in0=st_v,
                    scalar1=DVE_SCALE, scalar2=DVE_BIAS,
                    op0=mybir.AluOpType.mult, op1=mybir.AluOpType.add)

        # One flat loop over the global pair index: PE interleaves chunk c's
        # trailing zu/rs with chunk c+1's leading S^T so the exp ring never
        # drains at chunk boundaries.  zu/rs emission ups are capped near the
        # chunk end so each accumulator bank's last matmul lands just before
        # its ACT evacuation (up 3 / 11 of the next chunk) and the bank is
        # back in service for the next chunk's first accumulation (up 7 / 13).
        # zu/rs emission schedule.  Mid-chunks spread the 54 accumulation
        # matmuls linearly (~1.1 per up, inside PE's per-period slack) so
        # there is never a PE burst that stalls S^T production, while each
        # chunk's stream ends early enough in the next chunk (up 6 / 12) for
        # the ACT evacuations (zu copy up 8, rs ln up 14) to turn the bank
        # around before the next stream starts (up 11 / 17).  The final
        # chunk compresses instead: the exp engines are draining, so PE
        # bursts are free and the tail shortens.
        EXBUFS = 20              # ex ring depth >= max rs lag (17) + 3
        import collections as _cl
        sched_zu = _cl.defaultdict(list)
        sched_rs = _cl.defaultdict(list)
        for c in range(4):
            for s in range(PAIRS):
                if c < 3:
                    uz = 11 + (s * 49) // 53
                    ur = 17 + (s * 49) // 53
                else:
                    # steeper spread: still floored past the bank handoff,
                    # but ending by up 55/56 to shorten the drain tail
                    uz = 11 + (s * 44) // 53
                    ur = 17 + (s * 39) // 53
                sched_zu[PAIRS * c + uz].append((c, s))
                sched_rs[PAIRS * c + ur].append((c, s))

        zu_tiles, rs_tiles, ex_tiles, accs = {}, {}, {}, [0.0] * 4
        TOT = 4 * PAIRS
        for g in range(TOT + RLAG):
            c1, up = divmod(g, PAIRS)
            if c1 < 4:
                if up == 8 and "zu_p" in pend:
                    epi_zu()
                if up == 14 and "rs_p" in pend:
                    epi_rsln()
                if up == 20 and "rs_l" in pend:
                    epi_rsexp()
                if up == 26 and "outu_s" in pend and "bc" in pend:
                    epi_fin()
            if g < TOT:
                ch, s = c1, up
                w, off = CHW[ch], COFF[ch]
                quad = w <= 256
                if not quad or s % 2 == 0:
                    accs[ch] += ACT_FRAC[ch]
                    ua = accs[ch] >= 1.0
                    if ua:
                        accs[ch] -= 1.0
                if not quad:
                    stp = ppool.tile([C, 1024], F32, tag="st", bufs=3)
                    for j in range(2):
                        nc.tensor.matmul(stp[:, w * j:w * j + w],
                                         k3t(2 * s + j),
                                         q3[:, :, bass.ds(off, w)],
                                         start=True, stop=True,
                                         perf_mode=DR)
                    ex = spool.tile([C, 1024], FP8, tag="ex",
                                    bufs=EXBUFS)
                    do_exp(ua, ex[:, 0:2 * w], stp[:, 0:2 * w])
                    ex_tiles[(ch, s)] = (ex, 0)
                elif s % 2 == 0:
                    # quad: key tiles 2s..2s+3 in one slot, banks at
                    # [0:2w] and [512:512+2w]; single exp op of 4w elems
                    stp = ppool.tile([C, 1024], F32, tag="st", bufs=3)
                    for j in range(4):
                        base = w * j if j < 2 else 512 + w * (j - 2)
                        nc.tensor.matmul(stp[:, base:base + w],
                                         k3t(2 * s + j),
                                         q3[:, :, bass.ds(off, w)],
                                         start=True, stop=True,
                                         perf_mode=DR)
                    ex = spool.tile([C, 1024], FP8, tag="ex",
                                    bufs=EXBUFS)
                    st_v = stp.rearrange(
                        "p (b x) -> p b x", b=2)[:, :, 0:2 * w]
                    ex_v = ex[:, 0:4 * w].rearrange(
                        "p (b x) -> p b x", b=2)
                    do_exp(ua, ex_v, st_v)
                    ex_tiles[(ch, s)] = (ex, 0)
                    ex_tiles[(ch, s + 1)] = (ex, 2 * w)
            for (c, s) in sched_zu.get(g, []):
                w = CHW[c]
                if s == 0:
                    zu_tiles[c] = ppool.tile([C, 512], F32, tag="zu", name="zu")
                ex, xoff = ex_tiles[(c, s)]
                ex3 = ex[:, xoff:xoff + 2 * w].rearrange(
                    "p (b x) -> p b x", b=2)
                xk3 = v8T[:, bass.ds(256 * s, 256)].rearrange(
                    "p (b c) -> p b c", b=2)
                nc.tensor.matmul(zu_tiles[c][:, 0:w], xk3, ex3, perf_mode=DR,
                                 start=(s == 0), stop=(s == PAIRS - 1))
                if s == PAIRS - 1:
                    pend["zu_p"] = (zu_tiles[c], c)
            for (c, s) in sched_rs.get(g, []):
                w = CHW[c]
                if s == 0:
                    rs_tiles[c] = ppool.tile([1, 512], F32, tag="rs", name="rs")
                ex, xoff = ex_tiles.pop((c, s))
                ex3 = ex[:, xoff:xoff + 2 * w].rearrange(
                    "p (b x) -> p b x", b=2)
                nc.tensor.matmul(rs_tiles[c][:, 0:w], ones3, ex3,
                                 perf_mode=DR,
                                 start=(s == 0), stop=(s == PAIRS - 1))
                if s == PAIRS - 1:
                    pend["rs_p"] = (rs_tiles[c], c)

        # tail: final chunk's epilogue.  Per-engine issue order is what
        # matters: ACT zs -> ou, PE wv -> broadcast, DVE recip -> bc copy,
        # Pool rb -> fin; one y DMA (fixed HWDGE+latency cost dominates the
        # 192-col transfer).
        epi_zu()
        epi_recip_tail()
        epi_fin_tail(halves=2)

    nc.compile()
    _BUILD_CACHE[key] = nc
    return nc


def _prep_in_maps(x_q, x_kv, Wq, bq, Wk, bk, Wv, bv, gamma):
    f8 = ml_dtypes.float8_e4m3
    f32 = np.float32
    xq = np.asarray(x_q, f32).reshape(C, N)
    xkv = np.asarray(x_kv, f32).reshape(C, N)
    Wq = np.asarray(Wq, f32)
    bq = np.asarray(bq, f32)
    Wk = np.asarray(Wk, f32)
    bk = np.asarray(bk, f32)
    Wv = np.asarray(Wv, f32)
    bv = np.asarray(bv, f32)
    gamma = float(np.asarray(gamma, f32).reshape(()))

    # q/k projections (16xC) in f32 on host, straight into fp8 DR layout
    q = (Wq @ xq + bq[:, None]) * (0.25 * SQ)
    k = (Wk @ xkv + bk[:, None]) * SK
    q8 = np.clip(q, -224, 224).astype(f8)
    k8 = np.clip(k, -224, 224).astype(f8)
    k_s0 = np.concatenate([k8[0:8, 0:2048], k8[8:16, 0:2048]], axis=1)
    k_s1 = np.concatenate([k8[0:8, 2048:], k8[8:16, 2048:]], axis=1)

    # host-projected values: one fp8 quantization of gamma*(Wv@xkv) instead
    # of quantizing xkv AND Wv separately
    v_host = (Wv @ xkv) * gamma              # [C, N] f32
    am = float(np.abs(v_host).max())
    sv = float(2.0 ** np.floor(np.log2(224.0 / am))) if am > 0 else 1.0
    sv = min(max(sv, 2.0 ** -20), 2.0 ** 20)
    v8 = np.clip(v_host * sv, -224, 224).astype(f8)
    v8T = np.ascontiguousarray(
        v8.reshape(C, MT, 128).transpose(2, 1, 0).reshape(128, N))
    lnisv = np.full((1, 1), -math.log(sv), f32)
    isv1 = np.full((1, 1), 1.0 / sv, f32)
    resid = gamma * bv  # softmax rows sum to 1

    in_maps = []
    for c in range(NCORES):
        sl = slice(c * NQ, (c + 1) * NQ)
        q8c = q8[:, sl]
        qk = np.ascontiguousarray(np.concatenate(
            [q8c[0:8], q8c[8:16], k_s0, k_s1], axis=1))
        in_maps.append({
            "qk": qk, "v8T": v8T, "lnisv": lnisv, "isv1": isv1,
            "xq32": np.ascontiguousarray(xq[:, sl] + resid[:, None]),
        })
    return in_maps


def kernel(x_q, x_kv, Wq, bq, Wk, bk, Wv, bv, gamma):
    nc = build_nc(repeats=1)
    in_maps = _prep_in_maps(x_q, x_kv, Wq, bq, Wk, bk, Wv, bv, gamma)
    res = run_bass_kernel_spmd(nc, in_maps, list(range(NCORES)))
    out = np.concatenate([res.results[c]["y"] for c in range(NCORES)], axis=1)
    return out.reshape(1, C, D, H, W).astype(np.float32)



# revision 44
# speedup vs baseline: 1.0026x; 1.0016x over previous
"""CrossAttentionBlock Trainium2 kernel (v2).

Math:  q = (Wq xq + bq)/4; k = Wk xkv + bk; v = gamma*(Wv xkv + bv)
       P = softmax_rows(q^T k); out = x_q + v @ P^T   (gamma folded into v)

Strategy (8 cores, sequence-parallel: each core owns NQ=1728 queries vs all
N=13824 keys):
  * Host prep (layouts + the tiny 16xC q/k projections, f32, one fp8
    quantization): q8/k8 in fp8 DoubleRow layout ([8, 2N], virtual row
    r=p+8o, pow2-scaled 2^8/2^6); xkv transposed+tiled to fp8 key-major
    xkvT [128, N] ([key_local, 256s+128o+c]); Wv^T (gamma- and pow2-
    scaled) as a plain fp8 [128,128] stationary; x_q (+gamma*bv) in f32.
  * Device main loop: S^T pair-supertiles (2 key tiles x W query cols in
    one 2-bank PSUM slot, 3-slot ring) via fp8 DoubleRow matmuls; exp with
    deferred normalization, scaled exp(s)/32 so downstream sums stay in
    the e4m3 finite range (max 240).  ACT (table exp, scale/bias APs) and
    DVE (Schraudolph affine-to-uint8 e4m3 bit trick; uint8 saturation maps
    deep-negative scores to +0 instead of fp8 inf/nan) strictly alternate
    pairs - these two engines are the only PSUM readers and bound the
    kernel; regularity beats nominal weighted-RR capacity here.  exp'd
    tiles feed two accumulating DR matmuls: zu = sum_m xkvT_m ex_m (the
    UNPROJECTED attention numerator - no per-tile v evacuation ever) and
    rowsums rs = ones^T ex.  The last chunk (192 cols) runs 4-key-tile
    quad supertiles: one 768-elem exp op per slot amortizes the ~185ns
    fixed PSUM-access cost that dominates small ops.
  * Per-chunk epilogue, pipelined into the next chunk: zu -> fp8 (ACT),
    one plain fp8 [128x128] Wv matmul, evac with 1/sv scale-AP (ACT),
    reciprocal of rs (DVE), broadcast of 1/rs via replicate-DMA (0-stride
    free dim - no engine cost), Pool mul + residual add, store.  The small
    final chunk shortens the serial tail; its broadcast uses PE+DVE
    (lower latency than the replicate-DMA) and one single y DMA.
"""

import contextlib
import math

import numpy as np
import ml_dtypes

import concourse.bass as bass
import concourse.mybir as mybir
from concourse import bacc
from concourse.tile import TileContext
from concourse.bass_utils import run_bass_kernel_spmd

# The act-table placement pass resolves each activation to the first set
# containing its function, which splits {Exp, Identity/Copy} (set 0) from Ln
# (set 6) and thrashes ~1.3us LoadActFuncSet swaps mid-kernel.  This kernel
# only uses funcs that all live in natural_log_exp_and_others, so replace
# the pass with one pre-placed load of that set (walrus adopts pre-placed
# loads); set id keeps its original index so the walrus mapping is intact.
_orig_gat = bacc.get_activation_tables
_orig_iatl = getattr(bacc.Bacc.insert_act_table_loads, "_nl_orig",
                     bacc.Bacc.insert_act_table_loads)


def _single_act_load(self):
    used = {i.func for b in self.main_func.blocks
            for i in b.instructions if isinstance(i, mybir.InstActivation)}
    if not used:
        return
    tabs = _orig_gat(self.m.arch)
    names = list(tabs.keys())
    keep = "natural_log_exp_and_others"
    if keep not in tabs or not used <= tabs[keep]:
        return _orig_iatl(self)       # fall back to the stock pass
    ld = mybir.InstLoadActFuncSet(
        name=self.get_next_instruction_name(), ins=[], outs=[],
        act_func_set_id=names.index(keep))
    ld.engine = mybir.EngineType.Activation
    self.register_instruction(ld)
    # place directly before the first activation (same block) so it rides
    # the ACT queue during the input-DMA window instead of delaying any
    # block-entry barrier
    for blk in self.main_func.blocks:
        for idx, inst in enumerate(blk.instructions):
            if isinstance(inst, mybir.InstActivation):
                blk.instructions.insert(idx, ld)
                return
    self.main_func.blocks[0].instructions.insert(0, ld)


_single_act_load._nl_orig = _orig_iatl
bacc.Bacc.insert_act_table_loads = _single_act_load

F32 = mybir.dt.float32
BF16 = mybir.dt.bfloat16
FP8 = mybir.dt.float8e4
U8 = mybir.dt.uint8
AF = mybir.ActivationFunctionType
DR = mybir.MatmulPerfMode.DoubleRow

C = 128
RC = 16
D = H = W = 24
N = D * H * W            # 13824 keys
NCORES = 8
NQ = N // NCORES         # 1728 queries per core
MT = N // 128            # 108 key tiles
PAIRS = MT // 2          # 54 key-tile pairs
LAGP = 7                 # zu matmuls trail exp by this many pairs
RLAG = 13                # rs matmuls trail further: frees the rs bank later so
                         # the ACT-side rs evacuation can be spaced away from
                         # the other epilogue injections
CHW = [512, 512, 512, 192]
COFF = [0, 512, 1024, 1536]

SQ = 256.0               # q fp8 pow2 scale
SK = 64.0                # k fp8 pow2 scale
ES = 1.0 / (SQ * SK)     # exp input scale
LNDIV = math.log(32.0)   # ex = exp(s)/32 keeps zu inside fp8 range (max 240)
LOG2E = 1.4426950408889634
EXP8_SCALE = 8.0 * LOG2E
DVE_SCALE = EXP8_SCALE * ES
DVE_BIAS = 56.0 - 0.3 - 40.0   # e4m3 Schraudolph bias, -40 = the /32
# exp pair -> engine split.  Strict A,D,A,D alternation is the unique
# stall-free pattern on the 3-slot PSUM ring: same-engine ops land at slot
# distance 2 (never 3), so the exp(s) -> PE S^T(s+3) -> exp(s+3) rewrite
# chain (~350 ns) stays off every engine's back-to-back path.  The cycle is
# then 2 slots per max(ACT 1038, DVE 1192) = DVE-bound; ACT's 154 ns/period
# slack absorbs all epilogue PSUM reads (spaced injections below).
ACT_FRAC = [0.50, 0.50, 0.50, 0.50]
INJ = (8, 14, 24, 30)

_BUILD_CACHE: dict = {}


def _bcast_ap(src):
    """[1, w] AP -> [1, 128, w] with a 0-stride repeat dim (DMA replicate)."""
    ap = list(src.ap)
    return bass.AP(src.tensor, src.offset, [ap[0]] + [[0, 128]] + ap[1:])


def build_nc(repeats: int = 1):
    key = repeats
    if key in _BUILD_CACHE:
        return _BUILD_CACHE[key]

    nc = bacc.Bacc("TRN2", target_bir_lowering=False, debug=False,
                   num_devices=NCORES)
    # qk: [q_db (2*NQ) | k slice0 (2*2048) | k slice1 (2*(N-2048))], all in
    # DoubleRow o-major halves; one DMA covers q + the first 16 key tiles
    qk_dr = nc.dram_tensor("qk", [8, 2 * NQ + 2 * N], FP8,
                           kind="ExternalInput").ap()
    # v8T: host-projected values gamma*(Wv@xkv), pow2-scaled to fp8,
    # key-major tiled like the old xkvT.  Folding Wv on the host removes the
    # per-chunk wv matmul + zs/po evacuations AND one fp8 quantization.
    v8T_dr = nc.dram_tensor("v8T", [C, N], FP8, kind="ExternalInput").ap()
    lnisv_dr = nc.dram_tensor("lnisv", [1, 1], F32, kind="ExternalInput").ap()
    isv1_dr = nc.dram_tensor("isv1", [1, 1], F32, kind="ExternalInput").ap()
    xq_dr = nc.dram_tensor("xq32", [C, NQ], F32, kind="ExternalInput").ap()
    y = nc.dram_tensor("y", [C, NQ], F32, kind="ExternalOutput").ap()

    with TileContext(nc) as tc, contextlib.ExitStack() as ctx:
        cpool = ctx.enter_context(tc.tile_pool(name="consts", bufs=1))
        ppool = ctx.enter_context(tc.tile_pool(name="psum", bufs=1, space="PSUM"))
        spool = ctx.enter_context(tc.tile_pool(name="work", bufs=1))

        # ---- input DMAs, critical-path first ---------------------------------
        qk_sb = cpool.tile([8, 2 * NQ + 2 * N], FP8)
        Q0 = 2 * NQ          # 3456
        K0 = Q0 + 2 * 2048   # end of k slice0
        nc.sync.dma_start(qk_sb[:, 0:K0], qk_dr[:, 0:K0])
        nc.sync.dma_start(qk_sb[:, K0:], qk_dr[:, K0:])
        v8T = cpool.tile([C, N], FP8)
        for qq in range(4):
            sl = bass.ts(qq, N // 4)
            nc.sync.dma_start(v8T[:, sl], v8T_dr[:, sl])
        lnisv = cpool.tile([1, 1], F32)
        nc.sync.dma_start(lnisv[:], lnisv_dr[:])
        isv1 = cpool.tile([1, 1], F32)
        nc.sync.dma_start(isv1[:], isv1_dr[:])
        xq_sb = cpool.tile([C, NQ], F32)
        nc.sync.dma_start(xq_sb[:], xq_dr[:])

        ones_db = cpool.tile([C, 32], FP8)
        nc.gpsimd.memset(ones_db[:], 1.0)
        ones_row = cpool.tile([1, C], BF16)
        nc.gpsimd.memset(ones_row[:], 1.0)
        exp_bias = cpool.tile([C, 1], F32)
        nc.gpsimd.memset(exp_bias[:], -LNDIV)
        exp_scale = cpool.tile([C, 1], F32)
        nc.gpsimd.memset(exp_scale[:], ES)
        # dummy exp: hoists the ~1.3us activation-table load into the input
        # DMA window instead of delaying the first real ACT exp
        warm = cpool.tile([C, 1], F32)
        nc.scalar.activation(warm[:], exp_scale[:], AF.Exp)
        # PE p-state warmup: keep the tensor engine busy through the input
        # DMA window so the 3us continuous-execution ramp to full clock is
        # done before the first real S^T matmul
        dm = cpool.tile([8, 1024], FP8)
        nc.gpsimd.memset(dm[:], 1.0)
        dm3 = dm.rearrange("p (o x) -> p o x", o=2)
        wps = ppool.tile([C, 1024], F32, tag="st", bufs=3)
        for _ in range(8):
            nc.tensor.matmul(wps[:, 0:256], dm3[:, :, 0:128],
                             dm3[:, :, 0:256], start=True, stop=True,
                             perf_mode=DR)

        q3 = qk_sb[:, 0:Q0].rearrange("p (o x) -> p o x", o=2)
        k3a = qk_sb[:, Q0:K0].rearrange("p (o x) -> p o x", o=2)
        k3b = qk_sb[:, K0:].rearrange("p (o x) -> p o x", o=2)

        def k3t(t):
            if t < 16:
                return k3a[:, :, bass.ts(t, 128)]
            return k3b[:, :, bass.ts(t - 16, 128)]
        ones3 = ones_db.rearrange("p (b c) -> p b c", b=2)[:, :, 0:1]

        # ---- pipelined epilogue steps (run inside the NEXT chunk) ------------
        # All PSUM-reading epilogue ops go to ACT (DVE stays pure-exp): the
        # zu evac (now the final projected numerator, f32 straight to SBUF)
        # and the rowsum reciprocal via ln -> exp(-ln - ln sv), whose ln
        # doubles as the rs evacuation and whose bias folds in the host-side
        # fp8 scale.  Injections are spaced ~6 ups apart so ACT's per-period
        # slack covers them; the latency-critical tail keeps the DVE recip +
        # PE broadcast instead.
        pend = {}

        def epi_zu():
            zu, ch = pend.pop("zu_p")
            w = CHW[ch]
            ou = spool.tile([C, 512], F32, tag="outus", bufs=2)
            nc.scalar.copy(ou[:, 0:w], zu[:, 0:w])
            pend["outu_s"] = (ou, ch)

        def epi_rsln():
            rs, ch = pend.pop("rs_p")
            w = CHW[ch]
            rsl = spool.tile([1, 512], F32, tag="rsl", bufs=2)
            nc.scalar.activation(rsl[:, 0:w], rs[:, 0:w], AF.Ln)
            pend["rs_l"] = (rsl, ch)

        def epi_rsexp():
            rsl, ch = pend.pop("rs_l")
            w = CHW[ch]
            rsb = spool.tile([1, 512], F32, tag="rsb", bufs=2)
            nc.scalar.activation(rsb[:, 0:w], rsl[:, 0:w], AF.Exp,
                                 scale=-1.0, bias=lnisv[0:1, :])
            bc = spool.tile([C, 512], F32, tag="bc", bufs=2)
            nc.sync.dma_start(bc[:, 0:w], _bcast_ap(rsb[0:1, 0:w]))
            pend["bc"] = (bc, ch)

        def epi_recip_tail():
            rs, ch = pend.pop("rs_p")
            w = CHW[ch]
            recip = spool.tile([1, 512], F32, tag="recip", bufs=1)
            nc.vector.reciprocal_approx_fast(out=recip[:, 0:w], in_=rs[:, 0:w])
            # latency-critical tail: PE broadcast instead of replicate-DMA;
            # the rb copy folds in the 1/sv fp8 descale.  The broadcast PSUM
            # tile is consumed directly by the DVE fin mult - no bc copy.
            rb = spool.tile([1, 512], BF16, tag="rb", bufs=1)
            nc.gpsimd.tensor_scalar(out=rb[:, 0:w], in0=recip[:, 0:w],
                                    scalar1=isv1[0:1, :], scalar2=None,
                                    op0=mybir.AluOpType.mult)
            bp = ppool.tile([C, 1024], F32, tag="st", bufs=3)
            nc.tensor.matmul(bp[:, 0:w], ones_row[:], rb[:, 0:w],
                             start=True, stop=True)
            pend["bc_p"] = (bp, ch)

        def epi_fin(halves=1):
            ou, ch = pend.pop("outu_s")
            bc, _ = pend.pop("bc")
            w, off = CHW[ch], COFF[ch]
            hw = w // halves
            for h in range(halves):
                hsl = slice(h * hw, (h + 1) * hw)
                t1 = spool.tile([C, 512], F32, tag="t1", bufs=2)
                nc.gpsimd.tensor_mul(t1[:, 0:hw], ou[:, hsl], bc[:, hsl])
                res = spool.tile([C, 512], F32, tag="res", bufs=2)
                nc.gpsimd.tensor_add(res[:, 0:hw], t1[:, 0:hw],
                                     xq_sb[:, off + h * hw:off + (h + 1) * hw])
                nc.sync.dma_start(y[:, off + h * hw:off + (h + 1) * hw],
                                  res[:, 0:hw])

        def epi_fin_tail(halves=2):
            ou, ch = pend.pop("outu_s")
            bp, _ = pend.pop("bc_p")
            w, off = CHW[ch], COFF[ch]
            hw = w // halves
            for h in range(halves):
                hsl = slice(h * hw, (h + 1) * hw)
                t1 = spool.tile([C, 512], F32, tag="t1", bufs=2)
                nc.vector.tensor_mul(t1[:, 0:hw], ou[:, hsl], bp[:, hsl])
                res = spool.tile([C, 512], F32, tag="res", bufs=2)
                nc.gpsimd.tensor_add(res[:, 0:hw], t1[:, 0:hw],
                                     xq_sb[:, off + h * hw:off + (h + 1) * hw])
                nc.sync.dma_start(y[:, off + h * hw:off + (h + 1) * hw],
                                  res[:, 0:hw])

        # ---- main loop -------------------------------------------------------
        def do_exp(ua, ex_v, st_v):
            if ua:
                nc.scalar.activation(ex_v, st_v, AF.Exp,
                                     bias=exp_bias[:], scale=exp_scale[:])
            else:
                # uint8 out: conversion saturates at 0, so deep negative
                # scores clamp to fp8 +0 instead of the e4m3 inf/nan
                # patterns (bytes 0xF8..0xFF)
                nc.vector.tensor_scalar(
                    out=ex_v.bitcast(U8), in0=st_v,
                    scalar1=DVE_SCALE, scalar2=DVE_BIAS,
                    op0=mybir.AluOpType.mult, op1=mybir.AluOpType.add)

        # One flat loop over the global pair index: PE interleaves chunk c's
        # trailing zu/rs with chunk c+1's leading S^T so the exp ring never
        # drains at chunk boundaries.  zu/rs emission ups are capped near the
        # chunk end so each accumulator bank's last matmul lands just before
        # its ACT evacuation (up 3 / 11 of the next chunk) and the bank is
        # back in service for the next chunk's first accumulation (up 7 / 13).
        # zu/rs emission schedule.  Mid-chunks spread the 54 accumulation
        # matmuls linearly (~1.1 per up, inside PE's per-period slack) so
        # there is never a PE burst that stalls S^T production, while each
        # chunk's stream ends early enough in the next chunk (up 6 / 12) for
        # the ACT evacuations (zu copy up 8, rs ln up 14) to turn the bank
        # around before the next stream starts (up 11 / 17).  The final
        # chunk compresses instead: the exp engines are draining, so PE
        # bursts are free and the tail shortens.
        EXBUFS = 20              # ex ring depth >= max rs lag (17) + 3
        import collections as _cl
        sched_zu = _cl.defaultdict(list)
        sched_rs = _cl.defaultdict(list)
        for c in range(4):
            for s in range(PAIRS):
                if c < 3:
                    uz = 11 + (s * 49) // 53
                    ur = 17 + (s * 49) // 53
                else:
                    # steeper spread: still floored past the bank handoff,
                    # but ending by up 55/56 to shorten the drain tail
                    uz = 11 + (s * 44) // 53
                    ur = 17 + (s * 39) // 53
                sched_zu[PAIRS * c + uz].append((c, s))
                sched_rs[PAIRS * c + ur].append((c, s))

        zu_tiles, rs_tiles, ex_tiles, accs = {}, {}, {}, [0.0] * 4
        TOT = 4 * PAIRS
        for g in range(TOT + RLAG):
            c1, up = divmod(g, PAIRS)
            if c1 < 4:
                if up == INJ[0] and "zu_p" in pend:
                    epi_zu()
                if up == INJ[1] and "rs_p" in pend:
                    epi_rsln()
                if up == INJ[2] and "rs_l" in pend:
                    epi_rsexp()
                if up == INJ[3] and "outu_s" in pend and "bc" in pend:
                    epi_fin()
            if g < TOT:
                ch, s = c1, up
                w, off = CHW[ch], COFF[ch]
                quad = w <= 256
                if not quad or s % 2 == 0:
                    accs[ch] += ACT_FRAC[ch]
                    ua = accs[ch] >= 1.0
                    if ua:
                        accs[ch] -= 1.0
                if not quad:
                    stp = ppool.tile([C, 1024], F32, tag="st", bufs=3)
                    for j in range(2):
                        nc.tensor.matmul(stp[:, w * j:w * j + w],
                                         k3t(2 * s + j),
                                         q3[:, :, bass.ds(off, w)],
                                         start=True, stop=True,
                                         perf_mode=DR)
                    ex = spool.tile([C, 1024], FP8, tag="ex",
                                    bufs=EXBUFS)
                    do_exp(ua, ex[:, 0:2 * w], stp[:, 0:2 * w])
                    ex_tiles[(ch, s)] = (ex, 0)
                elif s % 2 == 0:
                    # quad: key tiles 2s..2s+3 in one slot, banks at
                    # [0:2w] and [512:512+2w]; single exp op of 4w elems
                    stp = ppool.tile([C, 1024], F32, tag="st", bufs=3)
                    for j in range(4):
                        base = w * j if j < 2 else 512 + w * (j - 2)
                        nc.tensor.matmul(stp[:, base:base + w],
                                         k3t(2 * s + j),
                                         q3[:, :, bass.ds(off, w)],
                                         start=True, stop=True,
                                         perf_mode=DR)
                    ex = spool.tile([C, 1024], FP8, tag="ex",
                                    bufs=EXBUFS)
                    st_v = stp.rearrange(
                        "p (b x) -> p b x", b=2)[:, :, 0:2 * w]
                    ex_v = ex[:, 0:4 * w].rearrange(
                        "p (b x) -> p b x", b=2)
                    do_exp(ua, ex_v, st_v)
                    ex_tiles[(ch, s)] = (ex, 0)
                    ex_tiles[(ch, s + 1)] = (ex, 2 * w)
            for (c, s) in sched_zu.get(g, []):
                w = CHW[c]
                if s == 0:
                    zu_tiles[c] = ppool.tile([C, 512], F32, tag="zu", name="zu")
                ex, xoff = ex_tiles[(c, s)]
                ex3 = ex[:, xoff:xoff + 2 * w].rearrange(
                    "p (b x) -> p b x", b=2)
                xk3 = v8T[:, bass.ds(256 * s, 256)].rearrange(
                    "p (b c) -> p b c", b=2)
                nc.tensor.matmul(zu_tiles[c][:, 0:w], xk3, ex3, perf_mode=DR,
                                 start=(s == 0), stop=(s == PAIRS - 1))
                if s == PAIRS - 1:
                    pend["zu_p"] = (zu_tiles[c], c)
            for (c, s) in sched_rs.get(g, []):
                w = CHW[c]
                if s == 0:
                    rs_tiles[c] = ppool.tile([1, 512], F32, tag="rs", name="rs")
                ex, xoff = ex_tiles.pop((c, s))
                ex3 = ex[:, xoff:xoff + 2 * w].rearrange(
                    "p (b x) -> p b x", b=2)
                nc.tensor.matmul(rs_tiles[c][:, 0:w], ones3, ex3,
                                 perf_mode=DR,
                                 start=(s == 0), stop=(s == PAIRS - 1))
                if s == PAIRS - 1:
                    pend["rs_p"] = (rs_tiles[c], c)

        # tail: final chunk's epilogue.  Per-engine issue order is what
        # matters: ACT zs -> ou, PE wv -> broadcast, DVE recip -> bc copy,
        # Pool rb -> fin; one y DMA (fixed HWDGE+latency cost dominates the
        # 192-col transfer).
        epi_zu()
        epi_recip_tail()
        epi_fin_tail(halves=2)

    nc.compile()
    _BUILD_CACHE[key] = nc
    return nc


def _prep_in_maps(x_q, x_kv, Wq, bq, Wk, bk, Wv, bv, gamma):
    f8 = ml_dtypes.float8_e4m3
    f32 = np.float32
    xq = np.asarray(x_q, f32).reshape(C, N)
    xkv = np.asarray(x_kv, f32).reshape(C, N)
    Wq = np.asarray(Wq, f32)
    bq = np.asarray(bq, f32)
    Wk = np.asarray(Wk, f32)
    bk = np.asarray(bk, f32)
    Wv = np.asarray(Wv, f32)
    bv = np.asarray(bv, f32)
    gamma = float(np.asarray(gamma, f32).reshape(()))

    # q/k projections (16xC) in f32 on host, straight into fp8 DR layout
    q = (Wq @ xq + bq[:, None]) * (0.25 * SQ)
    k = (Wk @ xkv + bk[:, None]) * SK
    q8 = np.clip(q, -224, 224).astype(f8)
    k8 = np.clip(k, -224, 224).astype(f8)
    k_s0 = np.concatenate([k8[0:8, 0:2048], k8[8:16, 0:2048]], axis=1)
    k_s1 = np.concatenate([k8[0:8, 2048:], k8[8:16, 2048:]], axis=1)

    # host-projected values: one fp8 quantization of gamma*(Wv@xkv) instead
    # of quantizing xkv AND Wv separately
    v_host = (Wv @ xkv) * gamma              # [C, N] f32
    am = float(np.abs(v_host).max())
    sv = float(2.0 ** np.floor(np.log2(224.0 / am))) if am > 0 else 1.0
    sv = min(max(sv, 2.0 ** -20), 2.0 ** 20)
    v8 = np.clip(v_host * sv, -224, 224).astype(f8)
    v8T = np.ascontiguousarray(
        v8.reshape(C, MT, 128).transpose(2, 1, 0).reshape(128, N))
    lnisv = np.full((1, 1), -math.log(sv), f32)
    isv1 = np.full((1, 1), 1.0 / sv, f32)
    resid = gamma * bv  # softmax rows sum to 1

    in_maps = []
    for c in range(NCORES):
        sl = slice(c * NQ, (c + 1) * NQ)
        q8c = q8[:, sl]
        qk = np.ascontiguousarray(np.concatenate(
            [q8c[0:8], q8c[8:16], k_s0, k_s1], axis=1))
        in_maps.append({
            "qk": qk, "v8T": v8T, "lnisv": lnisv, "isv1": isv1,
            "xq32": np.ascontiguousarray(xq[:, sl] + resid[:, None]),
        })
    return in_maps


def kernel(x_q, x_kv, Wq, bq, Wk, bk, Wv, bv, gamma):
    nc = build_nc(repeats=1)
    in_maps = _prep_in_maps(x_q, x_kv, Wq, bq, Wk, bk, Wv, bv, gamma)
    res = run_bass_kernel_spmd(nc, in_maps, list(range(NCORES)))
    out = np.concatenate([res.results[c]["y"] for c in range(NCORES)], axis=1)
    return out.reshape(1, C, D, H, W).astype(np.float32)



# revision 46
# speedup vs baseline: 1.0045x; 1.0019x over previous
"""CrossAttentionBlock Trainium2 kernel (v2).

Math:  q = (Wq xq + bq)/4; k = Wk xkv + bk; v = gamma*(Wv xkv + bv)
       P = softmax_rows(q^T k); out = x_q + v @ P^T   (gamma folded into v)

Strategy (8 cores, sequence-parallel: each core owns NQ=1728 queries vs all
N=13824 keys):
  * Host prep (layouts + the tiny 16xC q/k projections, f32, one fp8
    quantization): q8/k8 in fp8 DoubleRow layout ([8, 2N], virtual row
    r=p+8o, pow2-scaled 2^8/2^6); xkv transposed+tiled to fp8 key-major
    xkvT [128, N] ([key_local, 256s+128o+c]); Wv^T (gamma- and pow2-
    scaled) as a plain fp8 [128,128] stationary; x_q (+gamma*bv) in f32.
  * Device main loop: S^T pair-supertiles (2 key tiles x W query cols in
    one 2-bank PSUM slot, 3-slot ring) via fp8 DoubleRow matmuls; exp with
    deferred normalization, scaled exp(s)/32 so downstream sums stay in
    the e4m3 finite range (max 240).  ACT (table exp, scale/bias APs) and
    DVE (Schraudolph affine-to-uint8 e4m3 bit trick; uint8 saturation maps
    deep-negative scores to +0 instead of fp8 inf/nan) strictly alternate
    pairs - these two engines are the only PSUM readers and bound the
    kernel; regularity beats nominal weighted-RR capacity here.  exp'd
    tiles feed two accumulating DR matmuls: zu = sum_m xkvT_m ex_m (the
    UNPROJECTED attention numerator - no per-tile v evacuation ever) and
    rowsums rs = ones^T ex.  The last chunk (192 cols) runs 4-key-tile
    quad supertiles: one 768-elem exp op per slot amortizes the ~185ns
    fixed PSUM-access cost that dominates small ops.
  * Per-chunk epilogue, pipelined into the next chunk: zu -> fp8 (ACT),
    one plain fp8 [128x128] Wv matmul, evac with 1/sv scale-AP (ACT),
    reciprocal of rs (DVE), broadcast of 1/rs via replicate-DMA (0-stride
    free dim - no engine cost), Pool mul + residual add, store.  The small
    final chunk shortens the serial tail; its broadcast uses PE+DVE
    (lower latency than the replicate-DMA) and one single y DMA.
"""

import contextlib
import math

import numpy as np
import ml_dtypes

import concourse.bass as bass
import concourse.mybir as mybir
from concourse import bacc
from concourse.tile import TileContext
from concourse.bass_utils import run_bass_kernel_spmd

# The act-table placement pass resolves each activation to the first set
# containing its function, which splits {Exp, Identity/Copy} (set 0) from Ln
# (set 6) and thrashes ~1.3us LoadActFuncSet swaps mid-kernel.  This kernel
# only uses funcs that all live in natural_log_exp_and_others, so replace
# the pass with one pre-placed load of that set (walrus adopts pre-placed
# loads); set id keeps its original index so the walrus mapping is intact.
_orig_gat = bacc.get_activation_tables
_orig_iatl = getattr(bacc.Bacc.insert_act_table_loads, "_nl_orig",
                     bacc.Bacc.insert_act_table_loads)


def _single_act_load(self):
    used = {i.func for b in self.main_func.blocks
            for i in b.instructions if isinstance(i, mybir.InstActivation)}
    if not used:
        return
    tabs = _orig_gat(self.m.arch)
    names = list(tabs.keys())
    keep = "natural_log_exp_and_others"
    if keep not in tabs or not used <= tabs[keep]:
        return _orig_iatl(self)       # fall back to the stock pass
    ld = mybir.InstLoadActFuncSet(
        name=self.get_next_instruction_name(), ins=[], outs=[],
        act_func_set_id=names.index(keep))
    ld.engine = mybir.EngineType.Activation
    self.register_instruction(ld)
    # place directly before the first activation (same block) so it rides
    # the ACT queue during the input-DMA window instead of delaying any
    # block-entry barrier
    for blk in self.main_func.blocks:
        for idx, inst in enumerate(blk.instructions):
            if isinstance(inst, mybir.InstActivation):
                blk.instructions.insert(idx, ld)
                return
    self.main_func.blocks[0].instructions.insert(0, ld)


_single_act_load._nl_orig = _orig_iatl
bacc.Bacc.insert_act_table_loads = _single_act_load

F32 = mybir.dt.float32
BF16 = mybir.dt.bfloat16
FP8 = mybir.dt.float8e4
U8 = mybir.dt.uint8
AF = mybir.ActivationFunctionType
DR = mybir.MatmulPerfMode.DoubleRow

C = 128
RC = 16
D = H = W = 24
N = D * H * W            # 13824 keys
NCORES = 8
NQ = N // NCORES         # 1728 queries per core
MT = N // 128            # 108 key tiles
PAIRS = MT // 2          # 54 key-tile pairs
LAGP = 7                 # zu matmuls trail exp by this many pairs
RLAG = 13                # rs matmuls trail further: frees the rs bank later so
                         # the ACT-side rs evacuation can be spaced away from
                         # the other epilogue injections
CHW = [512, 512, 512, 192]
COFF = [0, 512, 1024, 1536]

SQ = 256.0               # q fp8 pow2 scale
SK = 64.0                # k fp8 pow2 scale
ES = 1.0 / (SQ * SK)     # exp input scale
LNDIV = math.log(32.0)   # ex = exp(s)/32 keeps zu inside fp8 range (max 240)
LOG2E = 1.4426950408889634
EXP8_SCALE = 8.0 * LOG2E
DVE_SCALE = EXP8_SCALE * ES
DVE_BIAS = 56.0 - 0.3 - 40.0   # e4m3 Schraudolph bias, -40 = the /32
# exp pair -> engine split.  Strict A,D,A,D alternation is the unique
# stall-free pattern on the 3-slot PSUM ring: same-engine ops land at slot
# distance 2 (never 3), so the exp(s) -> PE S^T(s+3) -> exp(s+3) rewrite
# chain (~350 ns) stays off every engine's back-to-back path.  The cycle is
# then 2 slots per max(ACT 1038, DVE 1192) = DVE-bound; ACT's 154 ns/period
# slack absorbs all epilogue PSUM reads (spaced injections below).
ACT_FRAC = [0.50, 0.50, 0.50, 0.50]
INJ = (8, 14, 24, 30)
SCH = (11, 49, 17, 49, 49, 49)

_BUILD_CACHE: dict = {}


def _bcast_ap(src):
    """[1, w] AP -> [1, 128, w] with a 0-stride repeat dim (DMA replicate)."""
    ap = list(src.ap)
    return bass.AP(src.tensor, src.offset, [ap[0]] + [[0, 128]] + ap[1:])


def build_nc(repeats: int = 1):
    key = repeats
    if key in _BUILD_CACHE:
        return _BUILD_CACHE[key]

    nc = bacc.Bacc("TRN2", target_bir_lowering=False, debug=False,
                   num_devices=NCORES)
    # qk: [q_db (2*NQ) | k slice0 (2*2048) | k slice1 (2*(N-2048))], all in
    # DoubleRow o-major halves; one DMA covers q + the first 16 key tiles
    qk_dr = nc.dram_tensor("qk", [8, 2 * NQ + 2 * N], FP8,
                           kind="ExternalInput").ap()
    # v8T: host-projected values gamma*(Wv@xkv), pow2-scaled to fp8,
    # key-major tiled like the old xkvT.  Folding Wv on the host removes the
    # per-chunk wv matmul + zs/po evacuations AND one fp8 quantization.
    v8T_dr = nc.dram_tensor("v8T", [C, N], FP8, kind="ExternalInput").ap()
    lnisv_dr = nc.dram_tensor("lnisv", [1, 1], F32, kind="ExternalInput").ap()
    isv1_dr = nc.dram_tensor("isv1", [1, 1], F32, kind="ExternalInput").ap()
    xq_dr = nc.dram_tensor("xq32", [C, NQ], F32, kind="ExternalInput").ap()
    y = nc.dram_tensor("y", [C, NQ], F32, kind="ExternalOutput").ap()

    with TileContext(nc) as tc, contextlib.ExitStack() as ctx:
        cpool = ctx.enter_context(tc.tile_pool(name="consts", bufs=1))
        ppool = ctx.enter_context(tc.tile_pool(name="psum", bufs=1, space="PSUM"))
        spool = ctx.enter_context(tc.tile_pool(name="work", bufs=1))

        # ---- input DMAs, critical-path first ---------------------------------
        qk_sb = cpool.tile([8, 2 * NQ + 2 * N], FP8)
        Q0 = 2 * NQ          # 3456
        K0 = Q0 + 2 * 2048   # end of k slice0
        nc.sync.dma_start(qk_sb[:, 0:K0], qk_dr[:, 0:K0])
        nc.sync.dma_start(qk_sb[:, K0:], qk_dr[:, K0:])
        v8T = cpool.tile([C, N], FP8)
        for qq in range(4):
            sl = bass.ts(qq, N // 4)
            nc.sync.dma_start(v8T[:, sl], v8T_dr[:, sl])
        lnisv = cpool.tile([1, 1], F32)
        nc.sync.dma_start(lnisv[:], lnisv_dr[:])
        isv1 = cpool.tile([1, 1], F32)
        nc.sync.dma_start(isv1[:], isv1_dr[:])
        xq_sb = cpool.tile([C, NQ], F32)
        nc.sync.dma_start(xq_sb[:], xq_dr[:])

        ones_db = cpool.tile([C, 32], FP8)
        nc.gpsimd.memset(ones_db[:], 1.0)
        ones_row = cpool.tile([1, C], BF16)
        nc.gpsimd.memset(ones_row[:], 1.0)
        exp_bias = cpool.tile([C, 1], F32)
        nc.gpsimd.memset(exp_bias[:], -LNDIV)
        exp_scale = cpool.tile([C, 1], F32)
        nc.gpsimd.memset(exp_scale[:], ES)
        # dummy exp: hoists the ~1.3us activation-table load into the input
        # DMA window instead of delaying the first real ACT exp
        warm = cpool.tile([C, 1], F32)
        nc.scalar.activation(warm[:], exp_scale[:], AF.Exp)
        # PE p-state warmup: keep the tensor engine busy through the input
        # DMA window so the 3us continuous-execution ramp to full clock is
        # done before the first real S^T matmul
        dm = cpool.tile([8, 1024], FP8)
        nc.gpsimd.memset(dm[:], 1.0)
        dm3 = dm.rearrange("p (o x) -> p o x", o=2)
        wps = ppool.tile([C, 1024], F32, tag="st", bufs=3)
        for _ in range(8):
            nc.tensor.matmul(wps[:, 0:256], dm3[:, :, 0:128],
                             dm3[:, :, 0:256], start=True, stop=True,
                             perf_mode=DR)

        q3 = qk_sb[:, 0:Q0].rearrange("p (o x) -> p o x", o=2)
        k3a = qk_sb[:, Q0:K0].rearrange("p (o x) -> p o x", o=2)
        k3b = qk_sb[:, K0:].rearrange("p (o x) -> p o x", o=2)

        def k3t(t):
            if t < 16:
                return k3a[:, :, bass.ts(t, 128)]
            return k3b[:, :, bass.ts(t - 16, 128)]
        ones3 = ones_db.rearrange("p (b c) -> p b c", b=2)[:, :, 0:1]

        # ---- pipelined epilogue steps (run inside the NEXT chunk) ------------
        # All PSUM-reading epilogue ops go to ACT (DVE stays pure-exp): the
        # zu evac (now the final projected numerator, f32 straight to SBUF)
        # and the rowsum reciprocal via ln -> exp(-ln - ln sv), whose ln
        # doubles as the rs evacuation and whose bias folds in the host-side
        # fp8 scale.  Injections are spaced ~6 ups apart so ACT's per-period
        # slack covers them; the latency-critical tail keeps the DVE recip +
        # PE broadcast instead.
        pend = {}

        def epi_zu():
            zu, ch = pend.pop("zu_p")
            w = CHW[ch]
            ou = spool.tile([C, 512], F32, tag="outus", bufs=2)
            nc.scalar.copy(ou[:, 0:w], zu[:, 0:w])
            pend["outu_s"] = (ou, ch)

        def epi_rsln():
            rs, ch = pend.pop("rs_p")
            w = CHW[ch]
            rsl = spool.tile([1, 512], F32, tag="rsl", bufs=2)
            nc.scalar.activation(rsl[:, 0:w], rs[:, 0:w], AF.Ln)
            pend["rs_l"] = (rsl, ch)

        def epi_rsexp():
            rsl, ch = pend.pop("rs_l")
            w = CHW[ch]
            rsb = spool.tile([1, 512], F32, tag="rsb", bufs=2)
            nc.scalar.activation(rsb[:, 0:w], rsl[:, 0:w], AF.Exp,
                                 scale=-1.0, bias=lnisv[0:1, :])
            bc = spool.tile([C, 512], F32, tag="bc", bufs=2)
            nc.sync.dma_start(bc[:, 0:w], _bcast_ap(rsb[0:1, 0:w]))
            pend["bc"] = (bc, ch)

        def epi_recip_tail():
            rs, ch = pend.pop("rs_p")
            w = CHW[ch]
            recip = spool.tile([1, 512], F32, tag="recip", bufs=1)
            nc.vector.reciprocal_approx_fast(out=recip[:, 0:w], in_=rs[:, 0:w])
            # latency-critical tail: PE broadcast instead of replicate-DMA;
            # the rb copy folds in the 1/sv fp8 descale.  The broadcast PSUM
            # tile is consumed directly by the DVE fin mult - no bc copy.
            rb = spool.tile([1, 512], BF16, tag="rb", bufs=1)
            nc.gpsimd.tensor_scalar(out=rb[:, 0:w], in0=recip[:, 0:w],
                                    scalar1=isv1[0:1, :], scalar2=None,
                                    op0=mybir.AluOpType.mult)
            bp = ppool.tile([C, 1024], F32, tag="st", bufs=3)
            nc.tensor.matmul(bp[:, 0:w], ones_row[:], rb[:, 0:w],
                             start=True, stop=True)
            pend["bc_p"] = (bp, ch)

        def epi_fin(halves=1):
            ou, ch = pend.pop("outu_s")
            bc, _ = pend.pop("bc")
            w, off = CHW[ch], COFF[ch]
            hw = w // halves
            for h in range(halves):
                hsl = slice(h * hw, (h + 1) * hw)
                t1 = spool.tile([C, 512], F32, tag="t1", bufs=2)
                nc.gpsimd.tensor_mul(t1[:, 0:hw], ou[:, hsl], bc[:, hsl])
                res = spool.tile([C, 512], F32, tag="res", bufs=2)
                nc.gpsimd.tensor_add(res[:, 0:hw], t1[:, 0:hw],
                                     xq_sb[:, off + h * hw:off + (h + 1) * hw])
                nc.sync.dma_start(y[:, off + h * hw:off + (h + 1) * hw],
                                  res[:, 0:hw])

        def epi_fin_tail(halves=2):
            ou, ch = pend.pop("outu_s")
            bp, _ = pend.pop("bc_p")
            w, off = CHW[ch], COFF[ch]
            hw = w // halves
            for h in range(halves):
                hsl = slice(h * hw, (h + 1) * hw)
                t1 = spool.tile([C, 512], F32, tag="t1", bufs=2)
                nc.vector.tensor_mul(t1[:, 0:hw], ou[:, hsl], bp[:, hsl])
                res = spool.tile([C, 512], F32, tag="res", bufs=2)
                nc.gpsimd.tensor_add(res[:, 0:hw], t1[:, 0:hw],
                                     xq_sb[:, off + h * hw:off + (h + 1) * hw])
                nc.sync.dma_start(y[:, off + h * hw:off + (h + 1) * hw],
                                  res[:, 0:hw])

        # ---- main loop -------------------------------------------------------
        def do_exp(ua, ex_v, st_v):
            if ua:
                nc.scalar.activation(ex_v, st_v, AF.Exp,
                                     bias=exp_bias[:], scale=exp_scale[:])
            else:
                # uint8 out: conversion saturates at 0, so deep negative
                # scores clamp to fp8 +0 instead of the e4m3 inf/nan
                # patterns (bytes 0xF8..0xFF)
                nc.vector.tensor_scalar(
                    out=ex_v.bitcast(U8), in0=st_v,
                    scalar1=DVE_SCALE, scalar2=DVE_BIAS,
                    op0=mybir.AluOpType.mult, op1=mybir.AluOpType.add)

        # One flat loop over the global pair index: PE interleaves chunk c's
        # trailing zu/rs with chunk c+1's leading S^T so the exp ring never
        # drains at chunk boundaries.  zu/rs emission ups are capped near the
        # chunk end so each accumulator bank's last matmul lands just before
        # its ACT evacuation (up 3 / 11 of the next chunk) and the bank is
        # back in service for the next chunk's first accumulation (up 7 / 13).
        # zu/rs emission schedule.  Mid-chunks spread the 54 accumulation
        # matmuls linearly (~1.1 per up, inside PE's per-period slack) so
        # there is never a PE burst that stalls S^T production, while each
        # chunk's stream ends early enough in the next chunk (up 6 / 12) for
        # the ACT evacuations (zu copy up 8, rs ln up 14) to turn the bank
        # around before the next stream starts (up 11 / 17).  The final
        # chunk compresses instead: the exp engines are draining, so PE
        # bursts are free and the tail shortens.
        EXBUFS = 20              # ex ring depth >= max rs lag (17) + 3
        import collections as _cl
        sched_zu = _cl.defaultdict(list)
        sched_rs = _cl.defaultdict(list)
        for c in range(4):
            for s in range(PAIRS):
                if c < 3:
                    uz = SCH[0] + (s * SCH[1]) // 53
                    ur = SCH[2] + (s * SCH[3]) // 53
                else:
                    # steeper spread: still floored past the bank handoff,
                    # but ending early to shorten the drain tail
                    uz = SCH[0] + (s * SCH[4]) // 53
                    ur = SCH[2] + (s * SCH[5]) // 53
                sched_zu[PAIRS * c + uz].append((c, s))
                sched_rs[PAIRS * c + ur].append((c, s))

        zu_tiles, rs_tiles, ex_tiles, accs = {}, {}, {}, [0.0] * 4
        TOT = 4 * PAIRS
        for g in range(TOT + RLAG):
            c1, up = divmod(g, PAIRS)
            if c1 < 4:
                if up == INJ[0] and "zu_p" in pend:
                    epi_zu()
                if up == INJ[1] and "rs_p" in pend:
                    epi_rsln()
                if up == INJ[2] and "rs_l" in pend:
                    epi_rsexp()
                if up == INJ[3] and "outu_s" in pend and "bc" in pend:
                    epi_fin()
            if g < TOT:
                ch, s = c1, up
                w, off = CHW[ch], COFF[ch]
                quad = w <= 256
                if not quad or s % 2 == 0:
                    accs[ch] += ACT_FRAC[ch]
                    ua = accs[ch] >= 1.0
                    if ua:
                        accs[ch] -= 1.0
                if not quad:
                    stp = ppool.tile([C, 1024], F32, tag="st", bufs=3)
                    for j in range(2):
                        nc.tensor.matmul(stp[:, w * j:w * j + w],
                                         k3t(2 * s + j),
                                         q3[:, :, bass.ds(off, w)],
                                         start=True, stop=True,
                                         perf_mode=DR)
                    ex = spool.tile([C, 1024], FP8, tag="ex",
                                    bufs=EXBUFS)
                    do_exp(ua, ex[:, 0:2 * w], stp[:, 0:2 * w])
                    ex_tiles[(ch, s)] = (ex, 0)
                elif s % 2 == 0:
                    # quad: key tiles 2s..2s+3 in one slot, banks at
                    # [0:2w] and [512:512+2w]; single exp op of 4w elems
                    stp = ppool.tile([C, 1024], F32, tag="st", bufs=3)
                    for j in range(4):
                        base = w * j if j < 2 else 512 + w * (j - 2)
                        nc.tensor.matmul(stp[:, base:base + w],
                                         k3t(2 * s + j),
                                         q3[:, :, bass.ds(off, w)],
                                         start=True, stop=True,
                                         perf_mode=DR)
                    ex = spool.tile([C, 1024], FP8, tag="ex",
                                    bufs=EXBUFS)
                    st_v = stp.rearrange(
                        "p (b x) -> p b x", b=2)[:, :, 0:2 * w]
                    ex_v = ex[:, 0:4 * w].rearrange(
                        "p (b x) -> p b x", b=2)
                    do_exp(ua, ex_v, st_v)
                    ex_tiles[(ch, s)] = (ex, 0)
                    ex_tiles[(ch, s + 1)] = (ex, 2 * w)
            for (c, s) in sched_zu.get(g, []):
                w = CHW[c]
                if s == 0:
                    zu_tiles[c] = ppool.tile([C, 512], F32, tag="zu", name="zu")
                ex, xoff = ex_tiles[(c, s)]
                ex3 = ex[:, xoff:xoff + 2 * w].rearrange(
                    "p (b x) -> p b x", b=2)
                xk3 = v8T[:, bass.ds(256 * s, 256)].rearrange(
                    "p (b c) -> p b c", b=2)
                nc.tensor.matmul(zu_tiles[c][:, 0:w], xk3, ex3, perf_mode=DR,
                                 start=(s == 0), stop=(s == PAIRS - 1))
                if s == PAIRS - 1:
                    pend["zu_p"] = (zu_tiles[c], c)
            for (c, s) in sched_rs.get(g, []):
                w = CHW[c]
                if s == 0:
                    rs_tiles[c] = ppool.tile([1, 512], F32, tag="rs", name="rs")
                ex, xoff = ex_tiles.pop((c, s))
                ex3 = ex[:, xoff:xoff + 2 * w].rearrange(
                    "p (b x) -> p b x", b=2)
                nc.tensor.matmul(rs_tiles[c][:, 0:w], ones3, ex3,
                                 perf_mode=DR,
                                 start=(s == 0), stop=(s == PAIRS - 1))
                if s == PAIRS - 1:
                    pend["rs_p"] = (rs_tiles[c], c)

        # tail: final chunk's epilogue.  Per-engine issue order is what
        # matters: ACT zs -> ou, PE wv -> broadcast, DVE recip -> bc copy,
        # Pool rb -> fin; one y DMA (fixed HWDGE+latency cost dominates the
        # 192-col transfer).
        epi_zu()
        epi_recip_tail()
        epi_fin_tail(halves=2)

    nc.compile()
    _BUILD_CACHE[key] = nc
    return nc


def _prep_in_maps(x_q, x_kv, Wq, bq, Wk, bk, Wv, bv, gamma):
    f8 = ml_dtypes.float8_e4m3
    f32 = np.float32
    xq = np.asarray(x_q, f32).reshape(C, N)
    xkv = np.asarray(x_kv, f32).reshape(C, N)
    Wq = np.asarray(Wq, f32)
    bq = np.asarray(bq, f32)
    Wk = np.asarray(Wk, f32)
    bk = np.asarray(bk, f32)
    Wv = np.asarray(Wv, f32)
    bv = np.asarray(bv, f32)
    gamma = float(np.asarray(gamma, f32).reshape(()))

    # q/k projections (16xC) in f32 on host, straight into fp8 DR layout
    q = (Wq @ xq + bq[:, None]) * (0.25 * SQ)
    k = (Wk @ xkv + bk[:, None]) * SK
    q8 = np.clip(q, -224, 224).astype(f8)
    k8 = np.clip(k, -224, 224).astype(f8)
    k_s0 = np.concatenate([k8[0:8, 0:2048], k8[8:16, 0:2048]], axis=1)
    k_s1 = np.concatenate([k8[0:8, 2048:], k8[8:16, 2048:]], axis=1)

    # host-projected values: one fp8 quantization of gamma*(Wv@xkv) instead
    # of quantizing xkv AND Wv separately
    v_host = (Wv @ xkv) * gamma              # [C, N] f32
    am = float(np.abs(v_host).max())
    sv = float(2.0 ** np.floor(np.log2(224.0 / am))) if am > 0 else 1.0
    sv = min(max(sv, 2.0 ** -20), 2.0 ** 20)
    v8 = np.clip(v_host * sv, -224, 224).astype(f8)
    v8T = np.ascontiguousarray(
        v8.reshape(C, MT, 128).transpose(2, 1, 0).reshape(128, N))
    lnisv = np.full((1, 1), -math.log(sv), f32)
    isv1 = np.full((1, 1), 1.0 / sv, f32)
    resid = gamma * bv  # softmax rows sum to 1

    in_maps = []
    for c in range(NCORES):
        sl = slice(c * NQ, (c + 1) * NQ)
        q8c = q8[:, sl]
        qk = np.ascontiguousarray(np.concatenate(
            [q8c[0:8], q8c[8:16], k_s0, k_s1], axis=1))
        in_maps.append({
            "qk": qk, "v8T": v8T, "lnisv": lnisv, "isv1": isv1,
            "xq32": np.ascontiguousarray(xq[:, sl] + resid[:, None]),
        })
    return in_maps


def kernel(x_q, x_kv, Wq, bq, Wk, bk, Wv, bv, gamma):
    nc = build_nc(repeats=1)
    in_maps = _prep_in_maps(x_q, x_kv, Wq, bq, Wk, bk, Wv, bv, gamma)
    res = run_bass_kernel_spmd(nc, in_maps, list(range(NCORES)))
    out = np.concatenate([res.results[c]["y"] for c in range(NCORES)], axis=1)
    return out.reshape(1, C, D, H, W).astype(np.float32)

